# revision 2
# baseline (speedup 1.0000x reference)
"""KV page-cache scatter update on 8 Trainium2 NeuronCores.

Semantics (matches the reference):
    kv_ev = interleave(new_k, new_v)          # [T, 2H, D], head axis k0,v0,k1,v1,...
    for i in range(K):
        kv_pages[t_pages[i], t_slots[i]] = kv_ev[i]
    return kv_pages

Sharding: kv_pages is split along the page axis across the 8 cores
(256 pages each).  The host partitions the valid tokens by destination
page group and hands each core a compacted, interleaved update block plus
flat destination row indices.  Each core:
  1. copies its 33.5MB page shard input -> output with large DRAM->DRAM DMAs
  2. loads its update rows into SBUF and scatters them into the output with
     an indirect (SWDGE) DMA using the destination row indices.
Destinations are unique (page,slot) pairs, so padding duplicates the last
valid row (identical concurrent writes are benign).
"""

import numpy as np

from concourse import bacc, bass, mybir, tile
from concourse.bass_utils import run_bass_kernel_spmd

# Problem geometry (hardcoded per contract).
P, S, HH, D = 2048, 16, 16, 128   # pages, slots/page, 2*kv_heads, head_dim
T = 2048                          # new tokens
NCORES = 8
PC = P // NCORES                  # pages per core
RC = PC * S                       # flat rows per core (4096)
RD = HH * D                       # row width in f32 (2048 = 8KB)

_PROGRAM_CACHE: dict[int, object] = {}
_LAST_IN_MAPS: list | None = None  # stashed for test.py's traced re-run


def _build_program(n_pad: int, repeats: int = 1):
    """Bass program: copy kv shard in->out, then scatter n_pad update rows.

    repeats>1 repeats the whole body (copy + scatter) for slope-based
    device timing; the graded kernel runs repeats=1.
    """
    nc = bacc.Bacc("TRN2", target_bir_lowering=False, debug=False)

    kv_in = nc.dram_tensor("kv_in", [RC, RD], mybir.dt.float32, kind="ExternalInput")
    upd = nc.dram_tensor("upd", [n_pad, RD], mybir.dt.float32, kind="ExternalInput")
    dest = nc.dram_tensor("dest", [n_pad, 1], mybir.dt.int32, kind="ExternalInput")
    kv_out = nc.dram_tensor("kv_out", [RC, RD], mybir.dt.float32, kind="ExternalOutput")

    n_blocks = n_pad // 128
    total_elems = RC * RD
    n_chunks = 8
    chunk_elems = total_elems // n_chunks
    # inner descriptor rows of 8192 f32 (32KB), well under the 64KB AP limit
    inner = 8192
    chunk_rows = chunk_elems // inner

    with tile.TileContext(nc) as tc:
        with tc.tile_pool(name="sbuf", bufs=max(2, n_blocks)) as pool:
            for _ in range(repeats):
                # stage update rows + dest indices into SBUF; issued on
                # gpsimd (SWDGE) so they overlap the copy without occupying
                # the HWDGE rings that stream the bulk chunks
                tiles = []
                for b in range(n_blocks):
                    utile = pool.tile([128, RD], mybir.dt.float32)
                    dtile = pool.tile([128, 1], mybir.dt.int32)
                    nc.gpsimd.dma_start(out=utile[:], in_=upd[b * 128:(b + 1) * 128, :])
                    nc.gpsimd.dma_start(out=dtile[:], in_=dest[b * 128:(b + 1) * 128, :])
                    tiles.append((utile, dtile))

                # bulk copy: 8 chunks x 4MB, DRAM->DRAM, alternating across
                # the two HWDGE rings (SP and ACT) so both FIFOs stream
                for i in range(n_chunks):
                    off = i * chunk_elems
                    src = bass.AP(kv_in, off, [[inner, chunk_rows], [1, inner]])
                    dst = bass.AP(kv_out, off, [[inner, chunk_rows], [1, inner]])
                    eng = nc.sync if i % 2 == 0 else nc.scalar
                    eng.dma_start(out=dst, in_=src)

                # scatter update rows into kv_out (serialized after the copy
                # by Tile's WAW tracking on kv_out; destinations are unique)
                for utile, dtile in tiles:
                    nc.gpsimd.indirect_dma_start(
                        out=kv_out[:],
                        out_offset=bass.IndirectOffsetOnAxis(ap=dtile[:, :1], axis=0),
                        in_=utile[:],
                        in_offset=None,
                    )

    nc.compile()
    return nc


def kernel(kv_pages, t_pages, t_slots, new_k, new_v, K):
    kv_pages = np.asarray(kv_pages)
    t_pages = np.asarray(t_pages)
    t_slots = np.asarray(t_slots)
    new_k = np.asarray(new_k)
    new_v = np.asarray(new_v)
    k_valid = int(np.asarray(K))

    out_dtype = kv_pages.dtype
    Tn, Hn, Dn = new_k.shape

    # interleave K/V along the head axis: [T, 2H, D] -> flat [T, RD]
    kv_ev = np.empty((Tn, 2 * Hn, Dn), dtype=out_dtype)
    kv_ev[:, 0::2, :] = new_k
    kv_ev[:, 1::2, :] = new_v
    kv_ev = kv_ev.reshape(Tn, 2 * Hn * Dn)

    tp = t_pages[:k_valid].astype(np.int64)
    ts = t_slots[:k_valid].astype(np.int64)
    core_of = tp // PC

    sels = [np.nonzero(core_of == c)[0] for c in range(NCORES)]
    counts = [len(s) for s in sels]
    n_pad = max(128, -(-max(counts) // 128) * 128)

    if n_pad not in _PROGRAM_CACHE:
        _PROGRAM_CACHE[n_pad] = _build_program(n_pad)
    nc = _PROGRAM_CACHE[n_pad]

    kv_flat = kv_pages.reshape(P * S, RD)
    in_maps = []
    for c in range(NCORES):
        sel = sels[c]
        n = counts[c]
        upd = np.empty((n_pad, RD), dtype=out_dtype)
        dest = np.empty((n_pad, 1), dtype=np.int32)
        if n > 0:
            upd[:n] = kv_ev[sel]
            dest[:n, 0] = (tp[sel] - c * PC) * S + ts[sel]
            upd[n:] = upd[n - 1]
            dest[n:, 0] = dest[n - 1, 0]
        else:
            # no updates for this core: rewrite row 0 with its original data
            upd[:] = kv_flat[c * RC]
            dest[:, 0] = 0
        in_maps.append({
            "kv_in": np.ascontiguousarray(kv_flat[c * RC:(c + 1) * RC]),
            "upd": upd,
            "dest": dest,
        })

    global _LAST_IN_MAPS
    _LAST_IN_MAPS = in_maps
    res = run_bass_kernel_spmd(nc, in_maps, core_ids=list(range(NCORES)))
    out = np.concatenate(
        [res.results[c]["kv_out"].reshape(PC, S, HH, D) for c in range(NCORES)],
        axis=0,
    )
    return out.astype(out_dtype, copy=False)



# revision 3
# speedup vs baseline: 119.9211x; 119.9211x over previous
"""KV page-cache scatter update on 8 Trainium2 NeuronCores — in-place.

Semantics (matches the reference):
    kv_ev = interleave(new_k, new_v)          # [T, 2H, D], head axis k0,v0,k1,v1,...
    for i in range(K):
        kv_pages[t_pages[i], t_slots[i]] = kv_ev[i]
    return kv_pages

Sharding: kv_pages is split along the page axis across the 8 cores
(256 pages each).  The host partitions the valid tokens by destination
page group and hands each core a compacted, interleaved update block plus
flat destination row indices.

Key optimization vs the copy-based version: the device never copies the
33.5MB page shard.  The kv_pages shard is passed as the *donated backing
buffer* of the ExternalOutput `kv_out` (the same input-output aliasing
mechanism bass2jax.run_bass_via_pjrt uses to zero-init partially-written
outputs: the output tensor's buffer IS the donated operand, so rows the
kernel does not write retain the original kv_pages data).  The device
work is only the scatter of the update rows:
  1. load the ~2MB of update rows + dest indices into SBUF (HWDGE)
  2. indirect-scatter (SWDGE) them into kv_out at the destination rows.
Destinations are unique (page,slot) pairs; padding duplicates the last
valid row (identical concurrent writes are benign).
"""

import numpy as np

from concourse import bacc, bass, bass2jax, mybir, tile

# Problem geometry (hardcoded per contract).
P, S, HH, D = 2048, 16, 16, 128   # pages, slots/page, 2*kv_heads, head_dim
T = 2048                          # new tokens
NCORES = 8
PC = P // NCORES                  # pages per core
RC = PC * S                       # flat rows per core (4096)
RD = HH * D                       # row width in f32 (2048 = 8KB)

_PROGRAM_CACHE: dict[int, object] = {}
_FN_CACHE: dict[int, object] = {}
_LAST_IN_MAPS: list | None = None  # stashed for test.py's bench re-run


def _scatter_body(nc, pool, upd, dest, kv_out, n_blocks):
    """One kernel body: load update rows + indices, scatter into kv_out."""
    for b in range(n_blocks):
        utile = pool.tile([128, RD], mybir.dt.float32)
        dtile = pool.tile([128, 1], mybir.dt.int32)
        # update rows on the two HWDGE rings (fast path); indices on
        # SWDGE (tiny, keeps the HWDGE rings streaming)
        eng = nc.sync if b % 2 == 0 else nc.scalar
        eng.dma_start(out=utile[:], in_=upd[b * 128:(b + 1) * 128, :])
        nc.gpsimd.dma_start(out=dtile[:], in_=dest[b * 128:(b + 1) * 128, :])
        nc.gpsimd.indirect_dma_start(
            out=kv_out[:],
            out_offset=bass.IndirectOffsetOnAxis(ap=dtile[:, :1], axis=0),
            in_=utile[:],
            in_offset=None,
        )


def _build_program(n_pad: int, repeats: int = 1):
    """Bass program: scatter n_pad update rows into kv_out (in-place shard).

    repeats>1 wraps the same body in a hardware loop (x8 unrolled) for
    slope-based device timing; the graded kernel runs repeats=1.
    """
    nc = bacc.Bacc("TRN2", target_bir_lowering=False, debug=False)

    upd = nc.dram_tensor("upd", [n_pad, RD], mybir.dt.float32, kind="ExternalInput")
    dest = nc.dram_tensor("dest", [n_pad, 1], mybir.dt.int32, kind="ExternalInput")
    kv_out = nc.dram_tensor("kv_out", [RC, RD], mybir.dt.float32, kind="ExternalOutput")

    n_blocks = n_pad // 128

    with tile.TileContext(nc) as tc:
        with tc.tile_pool(name="sbuf", bufs=max(4, 2 * n_blocks)) as pool:
            if repeats == 1:
                _scatter_body(nc, pool, upd, dest, kv_out, n_blocks)
            else:
                unroll = 8
                assert repeats % unroll == 0
                with tc.For_i(0, repeats // unroll):
                    for _ in range(unroll):
                        _scatter_body(nc, pool, upd, dest, kv_out, n_blocks)

    nc.compile()
    return nc


def _io_meta(nc):
    import jax

    partition_name = nc.partition_id_tensor.name if nc.partition_id_tensor else None
    in_names, out_names, out_avals = [], [], []
    for alloc in nc.m.functions[0].allocations:
        if not isinstance(alloc, mybir.MemoryLocationSet):
            continue
        name = alloc.memorylocations[0].name
        if alloc.kind == "ExternalInput":
            if name != partition_name:
                in_names.append(name)
        elif alloc.kind == "ExternalOutput":
            out_names.append(name)
            out_avals.append(
                jax.core.ShapedArray(
                    tuple(alloc.tensor_shape), mybir.dt.np(alloc.dtype)
                )
            )
    return in_names, out_names, out_avals, partition_name


def _make_fn(nc):
    """Jitted 8-core shard_map around the bass program.

    Mirrors bass2jax.run_bass_via_pjrt's multi-core path, except the
    donated output-backing operand is supplied by the caller (the kv_pages
    shards) instead of zeros, so unwritten output rows retain kv_pages.
    """
    import jax

    bass2jax.install_neuronx_cc_hook()
    in_names, out_names, out_avals, partition_name = _io_meta(nc)
    all_in_names = in_names + out_names
    if partition_name is not None:
        all_in_names = all_in_names + [partition_name]
    n_params = len(in_names)

    def _body(*args):
        operands = list(args)
        if partition_name is not None:
            operands.append(bass2jax.partition_id_tensor())
        outs = bass2jax._bass_exec_p.bind(
            *operands,
            out_avals=tuple(out_avals),
            in_names=tuple(all_in_names),
            out_names=tuple(out_names),
            lowering_input_output_aliases=(),
            sim_require_finite=True,
            sim_require_nnan=True,
            nc=nc,
        )
        return tuple(outs)

    devices = jax.devices()[:NCORES]
    assert len(devices) == NCORES
    mesh = bass2jax.Mesh(np.asarray(devices), ("core",))
    spec = bass2jax.PartitionSpec("core")
    n_ops = len(in_names) + len(out_names)  # partition id supplied inside _body
    donate = tuple(range(n_params, n_ops))
    fn = jax.jit(
        bass2jax.shard_map(
            _body,
            mesh=mesh,
            in_specs=(spec,) * n_ops,
            out_specs=(spec,) * len(out_names),
            check_rep=False,
        ),
        donate_argnums=donate,
        keep_unused=True,
    )
    return fn, in_names


def kernel(kv_pages, t_pages, t_slots, new_k, new_v, K):
    kv_pages = np.asarray(kv_pages)
    t_pages = np.asarray(t_pages)
    t_slots = np.asarray(t_slots)
    new_k = np.asarray(new_k)
    new_v = np.asarray(new_v)
    k_valid = int(np.asarray(K))

    out_dtype = kv_pages.dtype
    Tn, Hn, Dn = new_k.shape

    # interleave K/V along the head axis: [T, 2H, D] -> flat [T, RD]
    kv_ev = np.empty((Tn, 2 * Hn, Dn), dtype=out_dtype)
    kv_ev[:, 0::2, :] = new_k
    kv_ev[:, 1::2, :] = new_v
    kv_ev = kv_ev.reshape(Tn, 2 * Hn * Dn)

    tp = t_pages[:k_valid].astype(np.int64)
    ts = t_slots[:k_valid].astype(np.int64)
    core_of = tp // PC

    sels = [np.nonzero(core_of == c)[0] for c in range(NCORES)]
    counts = [len(s) for s in sels]
    n_pad = max(128, -(-max(counts) // 128) * 128)

    if n_pad not in _PROGRAM_CACHE:
        _PROGRAM_CACHE[n_pad] = _build_program(n_pad)
    nc = _PROGRAM_CACHE[n_pad]
    if n_pad not in _FN_CACHE:
        _FN_CACHE[n_pad] = _make_fn(nc)
    fn, in_names = _FN_CACHE[n_pad]

    kv_flat = np.ascontiguousarray(kv_pages.reshape(P * S, RD))
    in_maps = []
    for c in range(NCORES):
        sel = sels[c]
        n = counts[c]
        upd = np.empty((n_pad, RD), dtype=out_dtype)
        dest = np.empty((n_pad, 1), dtype=np.int32)
        if n > 0:
            # sort by destination row: ordered HBM writes, and padding
            # duplicates the last (highest) row
            order = np.argsort(tp[sel], kind="stable")
            sel = sel[order]
            upd[:n] = kv_ev[sel]
            dest[:n, 0] = (tp[sel] - c * PC) * S + ts[sel]
            upd[n:] = upd[n - 1]
            dest[n:, 0] = dest[n - 1, 0]
        else:
            # no updates for this core: rewrite row 0 with its original data
            upd[:] = kv_flat[c * RC]
            dest[:, 0] = 0
        in_maps.append({"upd": upd, "dest": dest})

    global _LAST_IN_MAPS
    _LAST_IN_MAPS = in_maps

    concat_in = [
        np.concatenate([m[name] for m in in_maps], axis=0) for name in in_names
    ]
    out_arrs = fn(*concat_in, kv_flat)
    out = np.asarray(out_arrs[0]).reshape(P, S, HH, D)
    return out.astype(out_dtype, copy=False)


# revision 6
# speedup vs baseline: 229.5745x; 1.9144x over previous
"""KV page-cache scatter update on 8 Trainium2 NeuronCores — in-place.

Semantics (matches the reference):
    kv_ev = interleave(new_k, new_v)          # [T, 2H, D], head axis k0,v0,k1,v1,...
    for i in range(K):
        kv_pages[t_pages[i], t_slots[i]] = kv_ev[i]
    return kv_pages

Sharding: kv_pages is split along the page axis across the 8 cores (256
pages each).  The host partitions the valid tokens by destination page
group and hands each core a compacted, interleaved update block plus
destination row indices.

Two key optimizations vs the copy-based version:

1. In-place output (no 33.5MB/core device copy).  bass2jax passes
   ExternalOutput names as extra custom-call operands and jit-donates
   them; the output tensor's DRAM buffer IS the donated operand (the
   mechanism run_bass_via_pjrt uses to zero-init partially-written
   outputs).  We pass the kv_pages shard itself as that operand, so rows
   the kernel does not write retain the original kv_pages data and the
   device work is only the ~2MB/core scatter.

2. Split output tensors.  An indirect (SWDGE) DMA scatters at most 128
   rows (one index per SBUF partition), so a core's ~230 update rows
   need two scatter ops.  With a single output tensor Tile serializes
   them on the WAW dependency (completion latency ~8us each).  Splitting
   the shard into two ExternalOutputs (upper/lower 128 pages) gives each
   scatter its own tensor: no WAW edge, the two latency chains overlap,
   and the body runs at the HBM roofline (~12us vs 18us serialized).

Padding rows carry an out-of-bounds index (dropped via bounds_check), so
only real updates are written.
"""

import numpy as np

from concourse import bacc, bass, bass2jax, mybir, tile

# Problem geometry (hardcoded per contract).
P, S, HH, D = 2048, 16, 16, 128   # pages, slots/page, 2*kv_heads, head_dim
T = 2048                          # new tokens
NCORES = 8
PC = P // NCORES                  # pages per core
RC = PC * S                       # flat rows per core (4096)
HC = RC // 2                      # rows per half-shard (2048)
RD = HH * D                       # row width in f32 (2048 = 8KB)

_PROGRAM_CACHE: dict[tuple, object] = {}
_FN_CACHE: dict[tuple, object] = {}
_LAST_IN_MAPS: list | None = None  # stashed for test.py's bench re-run


def _scatter_body(nc, pool, upd, dest, kv_a, kv_b, k_a, k_b):
    """One kernel body: load update rows + indices, scatter into kv_a/kv_b.

    Block j (128 rows) scatters to kv_a for j < k_a, else kv_b.  Indices
    are local to the half; bounds_check drops padding (index == HC).
    """
    k = k_a + k_b
    # all indices in one SWDGE load: dtile[p, j] <- dest[j*128 + p]
    dtile = pool.tile([128, k], mybir.dt.int32)
    nc.gpsimd.dma_start(out=dtile[:], in_=bass.AP(dest, 0, [[1, 128], [128, k]]))
    for j in range(k):
        utile = pool.tile([128, RD], mybir.dt.float32)
        eng = nc.sync if j % 2 == 0 else nc.scalar
        eng.dma_start(out=utile[:], in_=upd[j * 128:(j + 1) * 128, :])
        kv_h = kv_a if j < k_a else kv_b
        nc.gpsimd.indirect_dma_start(
            out=kv_h[:],
            out_offset=bass.IndirectOffsetOnAxis(ap=dtile[:, j:j + 1], axis=0),
            in_=utile[:],
            in_offset=None,
            bounds_check=HC - 1,
            oob_is_err=False,
        )


def _build_program(key: tuple, repeats: int = 1):
    """Bass program: scatter (k_a + k_b)*128 update rows into the two
    in-place half-shards.

    repeats>1 wraps the same body in a hardware loop (x8 unrolled) for
    slope-based device timing; the graded kernel runs repeats=1.
    """
    k_a, k_b = key
    n_pad = (k_a + k_b) * 128
    nc = bacc.Bacc("TRN2", target_bir_lowering=False, debug=False)

    upd = nc.dram_tensor("upd", [n_pad, RD], mybir.dt.float32, kind="ExternalInput")
    dest = nc.dram_tensor("dest", [n_pad, 1], mybir.dt.int32, kind="ExternalInput")
    kv_a = nc.dram_tensor("kv_a", [HC, RD], mybir.dt.float32, kind="ExternalOutput")
    kv_b = nc.dram_tensor("kv_b", [HC, RD], mybir.dt.float32, kind="ExternalOutput")

    with tile.TileContext(nc) as tc:
        with tc.tile_pool(name="sbuf", bufs=max(4, min(8, 24 // (k_a + k_b)))) as pool:
            if repeats == 1:
                _scatter_body(nc, pool, upd, dest, kv_a, kv_b, k_a, k_b)
            else:
                unroll = 8
                assert repeats % unroll == 0
                with tc.For_i(0, repeats // unroll):
                    for _ in range(unroll):
                        _scatter_body(nc, pool, upd, dest, kv_a, kv_b, k_a, k_b)

    nc.compile()
    return nc


def _io_meta(nc):
    import jax

    partition_name = nc.partition_id_tensor.name if nc.partition_id_tensor else None
    in_names, out_names, out_avals = [], [], []
    for alloc in nc.m.functions[0].allocations:
        if not isinstance(alloc, mybir.MemoryLocationSet):
            continue
        name = alloc.memorylocations[0].name
        if alloc.kind == "ExternalInput":
            if name != partition_name:
                in_names.append(name)
        elif alloc.kind == "ExternalOutput":
            out_names.append(name)
            out_avals.append(
                jax.core.ShapedArray(
                    tuple(alloc.tensor_shape), mybir.dt.np(alloc.dtype)
                )
            )
    return in_names, out_names, out_avals, partition_name


def _make_fn(nc):
    """Jitted 8-core shard_map around the bass program.

    Mirrors bass2jax.run_bass_via_pjrt's multi-core path, except the
    donated output-backing operands are supplied by the caller (the
    kv_pages half-shards) instead of zeros, so unwritten output rows
    retain kv_pages data.
    """
    import jax

    bass2jax.install_neuronx_cc_hook()
    in_names, out_names, out_avals, partition_name = _io_meta(nc)
    all_in_names = in_names + out_names
    if partition_name is not None:
        all_in_names = all_in_names + [partition_name]
    n_params = len(in_names)

    def _body(*args):
        operands = list(args)
        if partition_name is not None:
            operands.append(bass2jax.partition_id_tensor())
        outs = bass2jax._bass_exec_p.bind(
            *operands,
            out_avals=tuple(out_avals),
            in_names=tuple(all_in_names),
            out_names=tuple(out_names),
            lowering_input_output_aliases=(),
            sim_require_finite=True,
            sim_require_nnan=True,
            nc=nc,
        )
        return tuple(outs)

    devices = jax.devices()[:NCORES]
    assert len(devices) == NCORES
    mesh = bass2jax.Mesh(np.asarray(devices), ("core",))
    spec = bass2jax.PartitionSpec("core")
    n_ops = len(in_names) + len(out_names)  # partition id supplied inside _body
    donate = tuple(range(n_params, n_ops))
    fn = jax.jit(
        bass2jax.shard_map(
            _body,
            mesh=mesh,
            in_specs=(spec,) * n_ops,
            out_specs=(spec,) * len(out_names),
            check_rep=False,
        ),
        donate_argnums=donate,
        keep_unused=True,
    )
    return fn, in_names


def kernel(kv_pages, t_pages, t_slots, new_k, new_v, K):
    kv_pages = np.asarray(kv_pages)
    t_pages = np.asarray(t_pages)
    t_slots = np.asarray(t_slots)
    new_k = np.asarray(new_k)
    new_v = np.asarray(new_v)
    k_valid = int(np.asarray(K))

    out_dtype = kv_pages.dtype
    Tn, Hn, Dn = new_k.shape

    # interleave K/V along the head axis: [T, 2H, D] -> flat [T, RD]
    kv_ev = np.empty((Tn, 2 * Hn, Dn), dtype=out_dtype)
    kv_ev[:, 0::2, :] = new_k
    kv_ev[:, 1::2, :] = new_v
    kv_ev = kv_ev.reshape(Tn, 2 * Hn * Dn)

    tp = t_pages[:k_valid].astype(np.int64)
    ts = t_slots[:k_valid].astype(np.int64)
    rows = tp * S + ts                 # global flat row
    core_of = rows // RC

    # per-core, per-half selections (sorted by destination row)
    sels = []
    for c in range(NCORES):
        sel = np.nonzero(core_of == c)[0]
        sel = sel[np.argsort(rows[sel], kind="stable")]
        local = rows[sel] - c * RC
        cut = np.searchsorted(local, HC)
        sels.append((sel[:cut], sel[cut:]))

    k_a = max(1, max(-(-len(a) // 128) for a, _ in sels))
    k_b = max(1, max(-(-len(b) // 128) for _, b in sels))
    key = (k_a, k_b)
    n_pad = (k_a + k_b) * 128

    if key not in _PROGRAM_CACHE:
        _PROGRAM_CACHE[key] = _build_program(key)
    nc = _PROGRAM_CACHE[key]
    if key not in _FN_CACHE:
        _FN_CACHE[key] = _make_fn(nc)
    fn, in_names = _FN_CACHE[key]

    kv_flat = np.ascontiguousarray(kv_pages.reshape(P * S, RD))
    in_maps = []
    for c in range(NCORES):
        sel_a, sel_b = sels[c]
        upd = np.zeros((n_pad, RD), dtype=out_dtype)
        dest = np.full((n_pad, 1), HC, dtype=np.int32)  # pad = OOB (dropped)
        na, nb = len(sel_a), len(sel_b)
        upd[:na] = kv_ev[sel_a]
        dest[:na, 0] = rows[sel_a] - c * RC
        off = k_a * 128
        upd[off:off + nb] = kv_ev[sel_b]
        dest[off:off + nb, 0] = rows[sel_b] - c * RC - HC
        in_maps.append({"upd": upd, "dest": dest})

    global _LAST_IN_MAPS
    _LAST_IN_MAPS = in_maps

    concat_in = [
        np.concatenate([m[name] for m in in_maps], axis=0) for name in in_names
    ]
    halves = kv_flat.reshape(NCORES, RC, RD)
    kv_a_g = np.ascontiguousarray(halves[:, :HC]).reshape(NCORES * HC, RD)
    kv_b_g = np.ascontiguousarray(halves[:, HC:]).reshape(NCORES * HC, RD)
    out_a, out_b = fn(*concat_in, kv_a_g, kv_b_g)

    out = np.empty((NCORES, RC, RD), dtype=out_dtype)
    out[:, :HC] = np.asarray(out_a).reshape(NCORES, HC, RD)
    out[:, HC:] = np.asarray(out_b).reshape(NCORES, HC, RD)
    return out.reshape(P, S, HH, D).astype(out_dtype, copy=False)


# revision 11
# speedup vs baseline: 230.0461x; 1.0021x over previous
"""KV page-cache scatter update on 8 Trainium2 NeuronCores — in-place.

Semantics (matches the reference):
    kv_ev = interleave(new_k, new_v)          # [T, 2H, D], head axis k0,v0,k1,v1,...
    for i in range(K):
        kv_pages[t_pages[i], t_slots[i]] = kv_ev[i]
    return kv_pages

Sharding: kv_pages is split along the page axis across the 8 cores (256
pages each).  The host partitions the valid tokens by destination page
group and hands each core a compacted, interleaved update block plus
destination row indices.

Two key optimizations vs the copy-based version:

1. In-place output (no 33.5MB/core device copy).  bass2jax passes
   ExternalOutput names as extra custom-call operands and jit-donates
   them; the output tensor's DRAM buffer IS the donated operand (the
   mechanism run_bass_via_pjrt uses to zero-init partially-written
   outputs).  We pass the kv_pages shard itself as that operand, so rows
   the kernel does not write retain the original kv_pages data and the
   device work is only the ~2MB/core scatter.

2. Split output tensors.  An indirect (SWDGE) DMA scatters at most 128
   rows (one index per SBUF partition), so a core's ~230 update rows
   need two scatter ops.  With a single output tensor Tile serializes
   them on the WAW dependency (completion latency ~8us each).  Splitting
   the shard into two ExternalOutputs (upper/lower 128 pages) gives each
   scatter its own tensor: no WAW edge, the two latency chains overlap,
   and the body runs at the HBM roofline (~12us vs 18us serialized).

Padding rows carry an out-of-bounds index (dropped via bounds_check), so
only real updates are written.
"""

import numpy as np

from concourse import bacc, bass, bass2jax, mybir, tile

# Problem geometry (hardcoded per contract).
P, S, HH, D = 2048, 16, 16, 128   # pages, slots/page, 2*kv_heads, head_dim
T = 2048                          # new tokens
NCORES = 8
PC = P // NCORES                  # pages per core
RC = PC * S                       # flat rows per core (4096)
HC = RC // 2                      # rows per half-shard (2048)
RD = HH * D                       # row width in f32 (2048 = 8KB)

_PROGRAM_CACHE: dict[tuple, object] = {}
_FN_CACHE: dict[tuple, object] = {}
_LAST_IN_MAPS: list | None = None  # stashed for test.py's bench re-run


def _scatter_body(nc, pool, upd, dest, kv_a, kv_b, k_a, k_b, r_a, r_b):
    """One kernel body: load update rows + indices, scatter into kv_a/kv_b.

    Block j (up to 128 rows) scatters to kv_a for j < k_a, else kv_b.
    Indices are local to the half; bounds_check drops padding (index == HC).
    When a half fits one block (k == 1), only r rows are loaded/scattered
    (the max valid count over cores, rounded up to 16) — trims the padded
    read traffic.
    """
    k = k_a + k_b
    del r_a, r_b  # load trimming caused NRT_EXEC_UNIT_UNRECOVERABLE; disabled
    # all indices in one SWDGE load: dtile[p, j] <- dest[j*128 + p]
    dtile = pool.tile([128, k], mybir.dt.int32)
    nc.gpsimd.dma_start(out=dtile[:], in_=bass.AP(dest, 0, [[1, 128], [128, k]]))
    for j in range(k):
        utile = pool.tile([128, RD], mybir.dt.float32)
        eng = nc.sync if j % 2 == 0 else nc.scalar
        eng.dma_start(out=utile[:], in_=upd[j * 128:(j + 1) * 128, :])
        kv_h = kv_a if j < k_a else kv_b
        nc.gpsimd.indirect_dma_start(
            out=kv_h[:],
            out_offset=bass.IndirectOffsetOnAxis(ap=dtile[:, j:j + 1], axis=0),
            in_=utile[:],
            in_offset=None,
            bounds_check=HC - 1,
            oob_is_err=False,
        )


def _build_program(key: tuple, repeats: int = 1):
    """Bass program: scatter (k_a + k_b)*128 update rows into the two
    in-place half-shards.

    repeats>1 wraps the same body in a hardware loop (x8 unrolled) for
    slope-based device timing; the graded kernel runs repeats=1.
    """
    k_a, k_b, r_a, r_b = key
    n_pad = (k_a + k_b) * 128
    nc = bacc.Bacc("TRN2", target_bir_lowering=False, debug=False)

    upd = nc.dram_tensor("upd", [n_pad, RD], mybir.dt.float32, kind="ExternalInput")
    dest = nc.dram_tensor("dest", [n_pad, 1], mybir.dt.int32, kind="ExternalInput")
    kv_a = nc.dram_tensor("kv_a", [HC, RD], mybir.dt.float32, kind="ExternalOutput")
    kv_b = nc.dram_tensor("kv_b", [HC, RD], mybir.dt.float32, kind="ExternalOutput")

    with tile.TileContext(nc) as tc:
        with tc.tile_pool(name="sbuf", bufs=max(4, min(8, 24 // (k_a + k_b)))) as pool:
            if repeats == 1:
                _scatter_body(nc, pool, upd, dest, kv_a, kv_b, k_a, k_b, r_a, r_b)
            else:
                unroll = 8
                assert repeats % unroll == 0
                with tc.For_i(0, repeats // unroll):
                    for _ in range(unroll):
                        _scatter_body(nc, pool, upd, dest, kv_a, kv_b, k_a, k_b, r_a, r_b)

    nc.compile()
    return nc


def _io_meta(nc):
    import jax

    partition_name = nc.partition_id_tensor.name if nc.partition_id_tensor else None
    in_names, out_names, out_avals = [], [], []
    for alloc in nc.m.functions[0].allocations:
        if not isinstance(alloc, mybir.MemoryLocationSet):
            continue
        name = alloc.memorylocations[0].name
        if alloc.kind == "ExternalInput":
            if name != partition_name:
                in_names.append(name)
        elif alloc.kind == "ExternalOutput":
            out_names.append(name)
            out_avals.append(
                jax.core.ShapedArray(
                    tuple(alloc.tensor_shape), mybir.dt.np(alloc.dtype)
                )
            )
    return in_names, out_names, out_avals, partition_name


def _make_fn(nc):
    """Jitted 8-core shard_map around the bass program.

    Mirrors bass2jax.run_bass_via_pjrt's multi-core path, except the
    donated output-backing operands are supplied by the caller (the
    kv_pages half-shards) instead of zeros, so unwritten output rows
    retain kv_pages data.
    """
    import jax

    bass2jax.install_neuronx_cc_hook()
    in_names, out_names, out_avals, partition_name = _io_meta(nc)
    all_in_names = in_names + out_names
    if partition_name is not None:
        all_in_names = all_in_names + [partition_name]
    n_params = len(in_names)

    def _body(*args):
        operands = list(args)
        if partition_name is not None:
            operands.append(bass2jax.partition_id_tensor())
        outs = bass2jax._bass_exec_p.bind(
            *operands,
            out_avals=tuple(out_avals),
            in_names=tuple(all_in_names),
            out_names=tuple(out_names),
            lowering_input_output_aliases=(),
            sim_require_finite=True,
            sim_require_nnan=True,
            nc=nc,
        )
        return tuple(outs)

    devices = jax.devices()[:NCORES]
    assert len(devices) == NCORES
    mesh = bass2jax.Mesh(np.asarray(devices), ("core",))
    spec = bass2jax.PartitionSpec("core")
    n_ops = len(in_names) + len(out_names)  # partition id supplied inside _body
    donate = tuple(range(n_params, n_ops))
    fn = jax.jit(
        bass2jax.shard_map(
            _body,
            mesh=mesh,
            in_specs=(spec,) * n_ops,
            out_specs=(spec,) * len(out_names),
            check_rep=False,
        ),
        donate_argnums=donate,
        keep_unused=True,
    )
    return fn, in_names


def kernel(kv_pages, t_pages, t_slots, new_k, new_v, K):
    kv_pages = np.asarray(kv_pages)
    t_pages = np.asarray(t_pages)
    t_slots = np.asarray(t_slots)
    new_k = np.asarray(new_k)
    new_v = np.asarray(new_v)
    k_valid = int(np.asarray(K))

    out_dtype = kv_pages.dtype
    Tn, Hn, Dn = new_k.shape

    # interleave K/V along the head axis: [T, 2H, D] -> flat [T, RD]
    kv_ev = np.empty((Tn, 2 * Hn, Dn), dtype=out_dtype)
    kv_ev[:, 0::2, :] = new_k
    kv_ev[:, 1::2, :] = new_v
    kv_ev = kv_ev.reshape(Tn, 2 * Hn * Dn)

    tp = t_pages[:k_valid].astype(np.int64)
    ts = t_slots[:k_valid].astype(np.int64)
    rows = tp * S + ts                 # global flat row
    core_of = rows // RC

    # per-core, per-half selections (sorted by destination row)
    sels = []
    for c in range(NCORES):
        sel = np.nonzero(core_of == c)[0]
        sel = sel[np.argsort(rows[sel], kind="stable")]
        local = rows[sel] - c * RC
        cut = np.searchsorted(local, HC)
        sels.append((sel[:cut], sel[cut:]))

    max_a = max(len(a) for a, _ in sels)
    max_b = max(len(b) for _, b in sels)
    k_a = max(1, -(-max_a // 128))
    k_b = max(1, -(-max_b // 128))
    # single-block halves load only the valid rows (rounded up to 16)
    r_a = min(128, max(16, -(-max_a // 16) * 16))
    r_b = min(128, max(16, -(-max_b // 16) * 16))
    key = (k_a, k_b, r_a, r_b)
    n_pad = (k_a + k_b) * 128

    if key not in _PROGRAM_CACHE:
        _PROGRAM_CACHE[key] = _build_program(key)
    nc = _PROGRAM_CACHE[key]
    if key not in _FN_CACHE:
        _FN_CACHE[key] = _make_fn(nc)
    fn, in_names = _FN_CACHE[key]

    kv_flat = np.ascontiguousarray(kv_pages.reshape(P * S, RD))
    in_maps = []
    for c in range(NCORES):
        sel_a, sel_b = sels[c]
        upd = np.zeros((n_pad, RD), dtype=out_dtype)
        dest = np.full((n_pad, 1), HC, dtype=np.int32)  # pad = OOB (dropped)
        na, nb = len(sel_a), len(sel_b)
        upd[:na] = kv_ev[sel_a]
        dest[:na, 0] = rows[sel_a] - c * RC
        off = k_a * 128
        upd[off:off + nb] = kv_ev[sel_b]
        dest[off:off + nb, 0] = rows[sel_b] - c * RC - HC
        in_maps.append({"upd": upd, "dest": dest})

    global _LAST_IN_MAPS
    _LAST_IN_MAPS = in_maps

    concat_in = [
        np.concatenate([m[name] for m in in_maps], axis=0) for name in in_names
    ]
    halves = kv_flat.reshape(NCORES, RC, RD)
    kv_a_g = np.ascontiguousarray(halves[:, :HC]).reshape(NCORES * HC, RD)
    kv_b_g = np.ascontiguousarray(halves[:, HC:]).reshape(NCORES * HC, RD)
    out_a, out_b = fn(*concat_in, kv_a_g, kv_b_g)

    out = np.empty((NCORES, RC, RD), dtype=out_dtype)
    out[:, :HC] = np.asarray(out_a).reshape(NCORES, HC, RD)
    out[:, HC:] = np.asarray(out_b).reshape(NCORES, HC, RD)
    return out.reshape(P, S, HH, D).astype(out_dtype, copy=False)
